# revision 8
# baseline (speedup 1.0000x reference)
"""Bahdanau-attention scoring kernel for 8 TRN2 NeuronCores (fp8 DoubleRow).

Reference computation (S=2048, B=32, H=1024):
    cat    = concat([broadcast(hidden), enc], axis=2)          # [S,B,2H]
    alphas = tanh(einsum('sbk,hk->sbh', cat, W_attn) + b_attn) # [S,B,H]
    scores = einsum('sbh,h->sb', alphas, v)                    # [S,B]
    out    = softmax(scores.T, axis=1)[:, None, :]             # [B,1,S]

Split W_attn = [W1 | W2]: z[s,b,:] = W2 @ enc[s,b,:] + hp[b,:] with
hp[b,:] = W1 @ hidden[b,:] + b_attn computed once per batch.

Layout: h_out on partitions, s on the free dim.  The dominant matmul
(S*B*H*H MACs) runs in fp8 e4m3 DoubleRow mode - one instruction
contracts TWO 128-deep k-tiles, doubling PE throughput vs bf16.  W2 is
pre-scaled by 32 on the host so its entries clear e4m3's subnormal
floor; the 1/32 rides the ACT tanh's free scale operand.  hp[b] lands
as a per-partition bias AP on the same tanh, and the v-contraction
(over h = partitions) is a tiny K=128 PE matmul accumulating [1,512]
score chunks in PSUM - the Vector engine does only the softmax tail.

hp path: W1/hidden in fp8 e3m4 (4-bit mantissa, *128 pre-scale), with
w1 blocks stationary so hp comes out already h-on-partitions [128, 4].

Sharding: data-parallel over batch.  Core c handles batches 4c..4c+3.
"""

import sys

for _p in ("/opt/trn_rl_repo", "/root/.axon_site/_ro/trn_rl_repo"):
    if _p not in sys.path:
        sys.path.insert(0, _p)

import numpy as np
import ml_dtypes

import concourse.bass as bass  # noqa: F401  (bass must import before tile)
import concourse.mybir as mybir
import concourse.tile as tile
from concourse import bacc
from concourse.bass_utils import run_bass_kernel_spmd

S, B, H = 2048, 32, 1024
NCORES = 8
BL = B // NCORES          # batches per core (4)
P = 128                   # SBUF partitions
HT = H // P               # k-tiles over h_in (8)
SC = 512                  # s-chunk per enc DMA / psum tile
NSC = S // SC             # s chunks per batch row (4)
NHC = H // P              # h_out blocks of 128 (8)
KP = HT // 2              # DoubleRow k-pairs per z group (4)

W2SCALE = 32.0            # pre-scale so W2 clears e4m3's subnormal floor
W1SCALE = 128.0           # same for W1 in e3m4 (max |W1*128| ~ 14 < 15.5)

F8E4 = mybir.dt.float8e4
F8E3 = mybir.dt.float8e3
BF16 = mybir.dt.bfloat16
F32 = mybir.dt.float32
AFT = mybir.ActivationFunctionType
DR = mybir.MatmulPerfMode.DoubleRow

_nc_cache = None


def build():
    nc = bacc.Bacc()
    enc = nc.declare_dram_parameter("enc", [BL, H, S], F8E4, isOutput=False)
    w2t = nc.declare_dram_parameter("w2t", [H, H], F8E4, isOutput=False)
    w1t = nc.declare_dram_parameter("w1t", [H, H], F8E3, isOutput=False)
    hid = nc.declare_dram_parameter("hid", [H, BL], F8E3, isOutput=False)
    ba = nc.declare_dram_parameter("ba", [1, H], BF16, isOutput=False)
    vp = nc.declare_dram_parameter("v", [P, HT], BF16, isOutput=False)
    out = nc.declare_dram_parameter("out", [BL, S], F32, isOutput=True)

    with tile.TileContext(nc) as tc:
        with (
            tc.tile_pool(name="const", bufs=1) as cpool,
            tc.tile_pool(name="encp", bufs=4) as encp,
            tc.tile_pool(name="alp", bufs=10) as alp,
            tc.tile_pool(name="sxp", bufs=2) as sxp,
            tc.tile_pool(name="smallp", bufs=4) as smallp,
            tc.tile_pool(name="zps", bufs=4, space="PSUM") as zps,
            tc.tile_pool(name="scps", bufs=1, space="PSUM") as scps,
        ):
            # --- resident constants ---
            # small hp-path operands + W1 blocks on the ACT hwdge queue,
            # hc-major so hp(hc=0) can start ~1us in
            hid_sb = cpool.tile([P, HT, BL], F8E3)
            nc.scalar.dma_start(hid_sb[:], hid.rearrange("(t p) b -> p t b", p=P))
            ba_sb = cpool.tile([1, H], BF16)
            nc.scalar.dma_start(ba_sb[:], ba[:])
            v_sb = cpool.tile([P, HT], BF16)
            nc.scalar.dma_start(v_sb[:], vp[:])
            ones1 = cpool.tile([1, BL], BF16)
            nc.vector.memset(ones1[:], 1.0)
            # W2 rows on sync, enc chunk 0 on gpsimd: parallel DMA engines so
            # the first z group has both operands early
            w2_sb = cpool.tile([P, HT, H], F8E4)
            et0 = encp.tile([P, HT, SC], F8E4, tag="enc")
            for kt in range(HT):
                nc.sync.dma_start(w2_sb[:, kt, :], w2t[kt * P:(kt + 1) * P, :])
                nc.gpsimd.dma_start(et0[:, kt, :], enc[0, kt * P:(kt + 1) * P, 0:SC])

            w1_sb = cpool.tile([P, HT, H], F8E3)
            hp_sb = cpool.tile([P, HT, BL], F32)

            # hp[:, hc, b] = (W1 @ hidden[b] + b_attn)[hc*128:(hc+1)*128]
            # w1 block stationary -> output lands h-on-partitions, no
            # transpose.  One strided DMA descriptor per block, issued right
            # before its matmuls so the ACT queue interleaves issue and mul.
            def emit_hp(hc):
                nc.scalar.dma_start(
                    w1_sb[:, :, hc * P:(hc + 1) * P],
                    w1t[:, hc * P:(hc + 1) * P].rearrange("(t p) h -> p t h", p=P))
                hpp = zps.tile([P, BL], F32, tag="z", name=f"hp{hc}")
                for kt in range(HT):
                    nc.tensor.matmul(
                        hpp[:], w1_sb[:, kt, hc * P:(hc + 1) * P],
                        hid_sb[:, kt, :],
                        start=(kt == 0), stop=False)
                # + b_attn as a K=1 rank-1 update (ba_col (x) ones); ba is
                # pre-scaled by W1SCALE on the host so the 1/W1SCALE below
                # rescales the whole sum at once
                nc.tensor.matmul(
                    hpp[:], ba_sb[:, hc * P:(hc + 1) * P], ones1[:],
                    start=False, stop=True)
                nc.scalar.mul(hp_sb[:, hc, :], hpp[:], 1.0 / W1SCALE)

            # hp first: tiny matmuls warm the PE pstate while W2/enc stream in
            for hc in range(NHC):
                emit_hp(hc)

            # --- main loop ---
            first = True
            ci = 0
            for b in range(BL):
                # [1, 2048] spanning 4 psum banks; one exp drains it per batch
                scores = scps.tile([1, S], F32, tag="sc")
                for sc in range(NSC):
                    if first:
                        et = et0
                    else:
                        et = encp.tile([P, HT, SC], F8E4, tag="enc")
                        q = nc.gpsimd if ci % 2 == 0 else nc.sync
                        q.dma_start(
                            et[:],
                            enc[b, :, sc * SC:(sc + 1) * SC].rearrange(
                                "(t p) s -> p t s", p=P))
                    ci += 1
                    als = []
                    for hc in range(NHC):
                        z = zps.tile([P, SC], F32, tag="z")
                        for j in range(KP):
                            nc.tensor.matmul(
                                z[:],
                                w2_sb[:, 2 * j:2 * j + 2, hc * P:(hc + 1) * P],
                                et[:, 2 * j:2 * j + 2, :],
                                start=(j == 0), stop=(j == KP - 1),
                                perf_mode=DR)
                        al = alp.tile([P, SC], BF16, tag="al")
                        nc.scalar.activation(
                            al[:], z[:], AFT.Tanh,
                            bias=hp_sb[:, hc, b:b + 1], scale=1.0 / W2SCALE)
                        als.append(al)
                    first = False
                    # v-dots batched at chunk end: tanh(hc) overlaps the
                    # z(hc+1) matmuls, so none of these stall the PE
                    for hc, al in enumerate(als):
                        nc.tensor.matmul(
                            scores[:, sc * SC:(sc + 1) * SC],
                            v_sb[:, hc:hc + 1], al[:],
                            start=(hc == 0), stop=(hc == NHC - 1),
                            skip_group_check=True)
                # --- softmax row b (no max-sub: |scores| <= sum|v| ~ 26) ---
                exs = sxp.tile([1, S], F32, tag="ex")
                tot = smallp.tile([1, 1], F32, tag="tot")
                nc.scalar.activation(exs[:], scores[:], AFT.Exp, accum_out=tot[:])
                rec = smallp.tile([1, 1], F32, tag="rec")
                nc.vector.reciprocal(rec[:], tot[:])
                osb = sxp.tile([1, S], F32, tag="osb")
                nc.vector.tensor_scalar_mul(osb[:], exs[:], rec[:, 0:1])
                nc.scalar.dma_start(out[b:b + 1, :], osb[:])
    nc.compile()
    return nc


def _get_nc():
    global _nc_cache
    if _nc_cache is None:
        _nc_cache = build()
    return _nc_cache


def _prep_inputs(hidden, encoder_outputs, W_attn, b_attn, v):
    e4 = ml_dtypes.float8_e4m3
    e3 = ml_dtypes.float8_e3m4
    bf = ml_dtypes.bfloat16
    hidden = np.asarray(hidden, dtype=np.float32)
    encoder_outputs = np.asarray(encoder_outputs, dtype=np.float32)
    W_attn = np.asarray(W_attn, dtype=np.float32)
    b_attn = np.asarray(b_attn, dtype=np.float32)
    v = np.asarray(v, dtype=np.float32)

    W1 = W_attn[:, :H]
    W2 = W_attn[:, H:]
    w2t = np.clip(W2.T * W2SCALE, -240.0, 240.0).astype(e4)     # [H kin, H hout]
    w1t = np.clip(W1.T * W1SCALE, -15.5, 15.5).astype(e3)
    hid_t = np.clip(hidden[0].T, -15.5, 15.5).astype(e3)        # [H, B]
    ba = (b_attn.reshape(1, H) * W1SCALE).astype(bf)
    vpt = np.ascontiguousarray(v.reshape(HT, P).T).astype(bf)   # [P, HT]
    enc_t = encoder_outputs.transpose(1, 2, 0).astype(e4)       # [B, H, S]

    in_maps = []
    for c in range(NCORES):
        bsl = slice(c * BL, (c + 1) * BL)
        in_maps.append({
            "enc": np.ascontiguousarray(enc_t[bsl]),
            "w2t": w2t,
            "w1t": w1t,
            "hid": np.ascontiguousarray(hid_t[:, bsl]),
            "ba": ba,
            "v": vpt,
        })
    return in_maps


def kernel(hidden, encoder_outputs, W_attn, b_attn, v, _trace=False):
    in_maps = _prep_inputs(hidden, encoder_outputs, W_attn, b_attn, v)
    nc = _get_nc()
    res = run_bass_kernel_spmd(
        nc, in_maps, core_ids=list(range(NCORES)), trace=_trace,
    )
    parts = [res.results[c]["out"] for c in range(NCORES)]      # each [BL, S]
    full = np.concatenate(parts, axis=0)                        # [B, S]
    out = full[:, None, :].astype(np.float32)                   # [B, 1, S]
    if _trace:
        return out, res
    return out


# revision 11
# speedup vs baseline: 1.0419x; 1.0419x over previous
"""Bahdanau-attention scoring kernel for 8 TRN2 NeuronCores (fp8 DoubleRow).

Reference computation (S=2048, B=32, H=1024):
    cat    = concat([broadcast(hidden), enc], axis=2)          # [S,B,2H]
    alphas = tanh(einsum('sbk,hk->sbh', cat, W_attn) + b_attn) # [S,B,H]
    scores = einsum('sbh,h->sb', alphas, v)                    # [S,B]
    out    = softmax(scores.T, axis=1)[:, None, :]             # [B,1,S]

Split W_attn = [W1 | W2]: z[s,b,:] = W2 @ enc[s,b,:] + hp[b,:] with
hp[b,:] = W1 @ hidden[b,:] + b_attn computed once per batch.

Layout: h_out on partitions, s on the free dim.  The dominant matmul
(S*B*H*H MACs) runs in fp8 e4m3 DoubleRow mode - one instruction
contracts TWO 128-deep k-tiles, doubling PE throughput vs bf16.  W2 is
pre-scaled by 32 on the host so its entries clear e4m3's subnormal
floor; the 1/32 rides the ACT tanh's free scale operand.  hp[b] lands
as a per-partition bias AP on the same tanh, and the v-contraction
(over h = partitions) is a tiny K=128 PE matmul accumulating [1,512]
score chunks in PSUM - the Vector engine does only the softmax tail.

hp path: W1/hidden in fp8 e3m4 (4-bit mantissa, *128 pre-scale), with
w1 blocks stationary so hp comes out already h-on-partitions [128, 4].

Sharding: data-parallel over batch.  Core c handles batches 4c..4c+3.
"""

import sys

for _p in ("/opt/trn_rl_repo", "/root/.axon_site/_ro/trn_rl_repo"):
    if _p not in sys.path:
        sys.path.insert(0, _p)

import numpy as np
import ml_dtypes

import concourse.bass as bass  # noqa: F401  (bass must import before tile)
import concourse.mybir as mybir
import concourse.tile as tile
from concourse import bacc
from concourse.bass_utils import run_bass_kernel_spmd

S, B, H = 2048, 32, 1024
NCORES = 8
BL = B // NCORES          # batches per core (4)
P = 128                   # SBUF partitions
HT = H // P               # k-tiles over h_in (8)
SC = 512                  # s-chunk per enc DMA / psum tile
NSC = S // SC             # s chunks per batch row (4)
NHC = H // P              # h_out blocks of 128 (8)
KP = HT // 2              # DoubleRow k-pairs per z group (4)

W2SCALE = 32.0            # pre-scale so W2 clears e4m3's subnormal floor
W1SCALE = 128.0           # same for W1 in e3m4 (max |W1*128| ~ 14 < 15.5)

F8E4 = mybir.dt.float8e4
F8E3 = mybir.dt.float8e3
BF16 = mybir.dt.bfloat16
F32 = mybir.dt.float32
AFT = mybir.ActivationFunctionType
DR = mybir.MatmulPerfMode.DoubleRow

_nc_cache = None


def build():
    nc = bacc.Bacc()
    enc = nc.declare_dram_parameter("enc", [BL, H, S], F8E4, isOutput=False)
    w2t = nc.declare_dram_parameter("w2t", [H, H], F8E4, isOutput=False)
    w1t = nc.declare_dram_parameter("w1t", [H, H], F8E3, isOutput=False)
    hid = nc.declare_dram_parameter("hid", [H, BL], F8E3, isOutput=False)
    ba = nc.declare_dram_parameter("ba", [1, H], BF16, isOutput=False)
    vp = nc.declare_dram_parameter("v", [P, HT], BF16, isOutput=False)
    out = nc.declare_dram_parameter("out", [BL, S], F32, isOutput=True)

    with tile.TileContext(nc) as tc:
        with (
            tc.tile_pool(name="const", bufs=1) as cpool,
            tc.tile_pool(name="encp", bufs=4) as encp,
            tc.tile_pool(name="alp", bufs=20) as alp,
            tc.tile_pool(name="sxp", bufs=2) as sxp,
            tc.tile_pool(name="smallp", bufs=4) as smallp,
            tc.tile_pool(name="zps", bufs=4, space="PSUM") as zps,
            tc.tile_pool(name="scps", bufs=1, space="PSUM") as scps,
        ):
            # --- resident constants ---
            # small hp-path operands + W1 blocks on the ACT hwdge queue,
            # hc-major so hp(hc=0) can start ~1us in
            hid_sb = cpool.tile([P, HT, BL], F8E3)
            nc.scalar.dma_start(hid_sb[:], hid.rearrange("(t p) b -> p t b", p=P))
            ba_sb = cpool.tile([1, H], BF16)
            nc.scalar.dma_start(ba_sb[:], ba[:])
            v_sb = cpool.tile([P, HT], BF16)
            nc.scalar.dma_start(v_sb[:], vp[:])
            ones1 = cpool.tile([1, BL], BF16)
            nc.vector.memset(ones1[:], 1.0)
            # W2 rows on sync, enc chunk 0 on gpsimd: parallel DMA engines so
            # the first z group has both operands early
            w2_sb = cpool.tile([P, HT, H], F8E4)
            et0 = encp.tile([P, HT, SC], F8E4, tag="enc")
            for kt in range(HT):
                nc.sync.dma_start(w2_sb[:, kt, :], w2t[kt * P:(kt + 1) * P, :])
                nc.gpsimd.dma_start(et0[:, kt, :], enc[0, kt * P:(kt + 1) * P, 0:SC])

            # W1 blocks ride the gpsimd queue behind enc chunk 0, in parallel
            # with W2 on sync: one strided descriptor per h_out block so
            # hp(hc) can start as soon as its block lands
            w1_sb = cpool.tile([P, HT, H], F8E3)
            for hc in range(NHC):
                nc.gpsimd.dma_start(
                    w1_sb[:, :, hc * P:(hc + 1) * P],
                    w1t[:, hc * P:(hc + 1) * P].rearrange("(t p) h -> p t h", p=P))
            hp_sb = cpool.tile([P, HT, BL], F32)

            # hp[:, hc, b] = (W1 @ hidden[b] + b_attn)[hc*128:(hc+1)*128]
            # w1 block stationary -> output lands h-on-partitions, no
            # transpose
            def emit_hp(hc):
                hpp = zps.tile([P, BL], F32, tag="z", name=f"hp{hc}")
                for kt in range(HT):
                    nc.tensor.matmul(
                        hpp[:], w1_sb[:, kt, hc * P:(hc + 1) * P],
                        hid_sb[:, kt, :],
                        start=(kt == 0), stop=False)
                # + b_attn as a K=1 rank-1 update (ba_col (x) ones); ba is
                # pre-scaled by W1SCALE on the host so the 1/W1SCALE below
                # rescales the whole sum at once
                nc.tensor.matmul(
                    hpp[:], ba_sb[:, hc * P:(hc + 1) * P], ones1[:],
                    start=False, stop=True)
                nc.scalar.mul(hp_sb[:, hc, :], hpp[:], 1.0 / W1SCALE)

            # v-dots + (on the batch's last chunk) the softmax tail for one
            # chunk.  Deferred one chunk behind the z matmuls so every al is
            # long since produced - no PE stalls waiting on tanh
            def flush_chunk(pend):
                b, sc, scores, als = pend
                for hc, al in enumerate(als):
                    nc.tensor.matmul(
                        scores[:, sc * SC:(sc + 1) * SC],
                        v_sb[:, hc:hc + 1], al[:],
                        start=(hc == 0), stop=(hc == NHC - 1),
                        skip_group_check=True)
                if sc == NSC - 1:
                    # softmax row b (no max-sub: |scores| <= sum|v| ~ 26)
                    exs = sxp.tile([1, S], F32, tag="ex")
                    tot = smallp.tile([1, 1], F32, tag="tot")
                    nc.scalar.activation(
                        exs[:], scores[:], AFT.Exp, accum_out=tot[:])
                    rec = smallp.tile([1, 1], F32, tag="rec")
                    nc.vector.reciprocal(rec[:], tot[:])
                    osb = sxp.tile([1, S], F32, tag="osb")
                    nc.vector.tensor_scalar_mul(osb[:], exs[:], rec[:, 0:1])
                    nc.scalar.dma_start(out[b:b + 1, :], osb[:])

            # --- main loop ---
            first = True
            ci = 0
            pending = None
            for b in range(BL):
                # [1, 2048] spanning 4 psum banks; one exp drains it per batch
                scores = scps.tile([1, S], F32, tag="sc")
                for sc in range(NSC):
                    if first:
                        et = et0
                    else:
                        et = encp.tile([P, HT, SC], F8E4, tag="enc")
                        q = nc.gpsimd if ci % 2 == 0 else nc.sync
                        q.dma_start(
                            et[:],
                            enc[b, :, sc * SC:(sc + 1) * SC].rearrange(
                                "(t p) s -> p t s", p=P))
                    ci += 1
                    if pending is not None:
                        flush_chunk(pending)
                    als = []
                    for hc in range(NHC):
                        z = zps.tile([P, SC], F32, tag="z")
                        for j in range(KP):
                            nc.tensor.matmul(
                                z[:],
                                w2_sb[:, 2 * j:2 * j + 2, hc * P:(hc + 1) * P],
                                et[:, 2 * j:2 * j + 2, :],
                                start=(j == 0), stop=(j == KP - 1),
                                perf_mode=DR)
                        if first:
                            # interleaved with chunk-0 z groups: hp(hc) psum
                            # borrows the z pool, so pairing keeps the pool
                            # rotation deadlock-free
                            emit_hp(hc)
                        al = alp.tile([P, SC], BF16, tag="al")
                        nc.scalar.activation(
                            al[:], z[:], AFT.Tanh,
                            bias=hp_sb[:, hc, b:b + 1], scale=1.0 / W2SCALE)
                        als.append(al)
                    first = False
                    pending = (b, sc, scores, als)
            flush_chunk(pending)
    nc.compile()
    return nc


def _get_nc():
    global _nc_cache
    if _nc_cache is None:
        _nc_cache = build()
    return _nc_cache


def _prep_inputs(hidden, encoder_outputs, W_attn, b_attn, v):
    e4 = ml_dtypes.float8_e4m3
    e3 = ml_dtypes.float8_e3m4
    bf = ml_dtypes.bfloat16
    hidden = np.asarray(hidden, dtype=np.float32)
    encoder_outputs = np.asarray(encoder_outputs, dtype=np.float32)
    W_attn = np.asarray(W_attn, dtype=np.float32)
    b_attn = np.asarray(b_attn, dtype=np.float32)
    v = np.asarray(v, dtype=np.float32)

    W1 = W_attn[:, :H]
    W2 = W_attn[:, H:]
    w2t = np.clip(W2.T * W2SCALE, -240.0, 240.0).astype(e4)     # [H kin, H hout]
    w1t = np.clip(W1.T * W1SCALE, -15.5, 15.5).astype(e3)
    hid_t = np.clip(hidden[0].T, -15.5, 15.5).astype(e3)        # [H, B]
    ba = (b_attn.reshape(1, H) * W1SCALE).astype(bf)
    vpt = np.ascontiguousarray(v.reshape(HT, P).T).astype(bf)   # [P, HT]
    enc_t = encoder_outputs.transpose(1, 2, 0).astype(e4)       # [B, H, S]

    in_maps = []
    for c in range(NCORES):
        bsl = slice(c * BL, (c + 1) * BL)
        in_maps.append({
            "enc": np.ascontiguousarray(enc_t[bsl]),
            "w2t": w2t,
            "w1t": w1t,
            "hid": np.ascontiguousarray(hid_t[:, bsl]),
            "ba": ba,
            "v": vpt,
        })
    return in_maps


def kernel(hidden, encoder_outputs, W_attn, b_attn, v, _trace=False):
    in_maps = _prep_inputs(hidden, encoder_outputs, W_attn, b_attn, v)
    nc = _get_nc()
    res = run_bass_kernel_spmd(
        nc, in_maps, core_ids=list(range(NCORES)), trace=_trace,
    )
    parts = [res.results[c]["out"] for c in range(NCORES)]      # each [BL, S]
    full = np.concatenate(parts, axis=0)                        # [B, S]
    out = full[:, None, :].astype(np.float32)                   # [B, 1, S]
    if _trace:
        return out, res
    return out
